# revision 3
# baseline (speedup 1.0000x reference)
"""MultiHeadAttention (no-transpose head reshape) on 8 TRN2 NeuronCores.

The reference reshapes [B,S,D] -> [B,H,S',dk] WITHOUT transposing, so
"head h" of batch b is exactly rows [128h, 128h+128) of x viewed as 2048
pseudo-tokens of dim 64.  That makes the whole problem data-parallel over
the 32 (b,h) pairs: each of the 8 cores owns 4 contiguous 128-row blocks
(512 rows) of one batch and needs no communication at all.

Per-core pipeline (bf16 matmuls, f32 accumulation):
  1. x [512,1024] -> xT bf16 via DMA transpose; weights cast to bf16.
  2. QT/KT = (x@W)^T via weight-stationary matmuls; V = x@Wv natural.
  3. Reorg into per-head QhT/KhT [64,2048] (DVE) and Vh chunks [128,65]
     (DMA) with a ones column appended to V.
  4. Attention per head: scoresT[k,q] = KhT^T@QhT -> exp on ACT (scale
     1/8 fused) -> probsT bf16 -> ctxT[65,q] += Vh1^T@probsT, where row 64
     accumulates the softmax denominators (ones column trick).
  5. Normalize + reorg to cbT (out-proj lhsT layout) fused in DVE mults.
  6. out = cb @ Wo + bo -> DRAM.
"""

import sys

if "/opt/trn_rl_repo" not in sys.path:
    sys.path.insert(0, "/opt/trn_rl_repo")

import numpy as np

import concourse.bacc as bacc
import concourse.mybir as mybir
import concourse.tile as tile
from concourse.bass_utils import run_bass_kernel_spmd

F32 = mybir.dt.float32
BF16 = mybir.dt.bfloat16
AF = mybir.ActivationFunctionType
ALU = mybir.AluOpType

N_CORES = 8
D = 1024
ROWS = 512          # rows of x per core
T = 4               # head-blocks per core
NJ = 8              # 128-feature chunks of D
DK = 64
S_PSEUDO = 2048     # pseudo-sequence length per head
KCG = 4             # k-chunks per exp group


def _emit(nc, tc, pools):
    persist = pools["persist"]
    stage = pools["stage"]
    psum_pp = pools["psum_pp"]
    psum_s = pools["psum_s"]
    psum_ctx = pools["psum_ctx"]

    x_d = nc.dram_tensor("x", [ROWS, D], F32, kind="ExternalInput")
    w_d = {}
    b_d = {}
    for w in ("q", "k", "v", "o"):
        w_d[w] = nc.dram_tensor(f"W{w}", [D, D], F32, kind="ExternalInput")
        b_d[w] = nc.dram_tensor(f"b{w}", [D], F32, kind="ExternalInput")
    out_d = nc.dram_tensor("out", [ROWS, D], F32, kind="ExternalOutput")

    # ---- persistent SBUF tensors ----
    xT = persist.tile([128, NJ, ROWS], BF16, name="xT")
    Wbf = {w: persist.tile([128, NJ, D], BF16, name=f"W{w}bf") for w in ("q", "k", "v", "o")}
    QT = persist.tile([128, NJ, ROWS], BF16, name="QT")
    KT = persist.tile([128, NJ, ROWS], BF16, name="KT")
    Vn = persist.tile([128, T, D], BF16, name="Vn")
    QhT = persist.tile([128, 2, S_PSEUDO], BF16, name="QhT")
    KhT = persist.tile([128, 2, S_PSEUDO], BF16, name="KhT")
    Vh = persist.tile([128, T, 16, DK + 1], BF16, name="Vh")
    cbT = persist.tile([128, T, NJ, 128], BF16, name="cbT")
    bq_sb = persist.tile([128, NJ], F32, name="bq_sb")
    bk_sb = persist.tile([128, NJ], F32, name="bk_sb")
    bvB = persist.tile([128, D], F32, name="bvB")
    boB = persist.tile([128, D], F32, name="boB")

    # ---- stage 0: loads, casts, transposes ----
    # x: load f32, cast to bf16, DMA-transpose 128x128 blocks into xT.
    for t in range(T):
        xs = stage.tile([128, D], F32, tag="xstage")
        nc.sync.dma_start(out=xs[:], in_=x_d[t * 128:(t + 1) * 128, :])
        xb = stage.tile([128, D], BF16, tag="xbf")
        nc.vector.tensor_copy(out=xb[:], in_=xs[:])
        for i in range(NJ):
            nc.sync.dma_start_transpose(
                out=xT[:, i, t * 128:(t + 1) * 128],
                in_=xb[:, i * 128:(i + 1) * 128],
            )

    # biases
    nc.sync.dma_start(out=bq_sb[:], in_=b_d["q"].ap().rearrange("(j p) -> p j", p=128))
    nc.sync.dma_start(out=bk_sb[:], in_=b_d["k"].ap().rearrange("(j p) -> p j", p=128))
    nc.sync.dma_start(out=bvB[0:1, :], in_=b_d["v"].ap().unsqueeze(0))
    nc.gpsimd.partition_broadcast(bvB[:], bvB[0:1, :])
    nc.sync.dma_start(out=boB[0:1, :], in_=b_d["o"].ap().unsqueeze(0))
    nc.gpsimd.partition_broadcast(boB[:], boB[0:1, :])

    # weights: Wq, Wk, Wv needed first; Wo last.  Cast on GPSIMD to keep
    # DVE free for the reorg copies.
    for w in ("q", "k", "v", "o"):
        for i in range(NJ):
            ws = stage.tile([128, D], F32, tag="wstage")
            nc.sync.dma_start(out=ws[:], in_=w_d[w][i * 128:(i + 1) * 128, :])
            nc.gpsimd.tensor_copy(out=Wbf[w][:, i, :], in_=ws[:])

    # ---- stage 1: projections ----
    # QT/KT: transposed outputs, weight chunks stationary, xT streams.
    for wname, bsb, dst in (("q", bq_sb, QT), ("k", bk_sb, KT)):
        for j in range(NJ):
            ps = psum_pp.tile([128, ROWS], F32, tag="pp")
            for i in range(NJ):
                nc.tensor.matmul(
                    ps[:],
                    lhsT=Wbf[wname][:, i, j * 128:(j + 1) * 128],
                    rhs=xT[:, i, :],
                    start=(i == 0),
                    stop=(i == NJ - 1),
                )
            nc.scalar.activation(dst[:, j, :], ps[:], AF.Identity, bias=bsb[:, j:j + 1])

    # V natural: xT chunks stationary, Wv streams.
    for t in range(T):
        for h in range(2):
            ps = psum_pp.tile([128, ROWS], F32, tag="pp")
            for i in range(NJ):
                nc.tensor.matmul(
                    ps[:],
                    lhsT=xT[:, i, t * 128:(t + 1) * 128],
                    rhs=Wbf["v"][:, i, h * 512:(h + 1) * 512],
                    start=(i == 0),
                    stop=(i == NJ - 1),
                )
            nc.vector.tensor_tensor(
                out=Vn[:, t, h * 512:(h + 1) * 512],
                in0=ps[:],
                in1=bvB[:, h * 512:(h + 1) * 512],
                op=ALU.add,
            )

    # ---- stage 2: head-layout reorgs ----
    # QhT/KhT: QhT[d', r*16+c] = QT[c*64+d', t*128+r]
    for srcT, dsthT in ((QT, QhT), (KT, KhT)):
        for t in range(T):
            p, eh = t // 2, t % 2
            for ce in range(2):
                src = srcT[ce * 64:(ce + 1) * 64, :, t * 128:(t + 1) * 128]
                dst = dsthT[eh * 64:(eh + 1) * 64, p, :].rearrange(
                    "p (r c) -> p c r", c=16
                )[:, ce::2, :]
                nc.vector.tensor_copy(out=dst, in_=src)

    # Vh: ones column then V data; Vh[(rr*16+c), t, kc, d'] = V[t*128+8kc+rr, c*64+d']
    nc.gpsimd.memset(Vh[:], 1.0)
    for t in range(T):
        for kc in range(16):
            nc.sync.dma_start(
                out=Vh[:, t, kc, 0:DK],
                in_=Vn[8 * kc:8 * kc + 8, t, :].rearrange("p (c d) -> p c d", c=16),
            )

    # ---- stage 3: attention ----
    for t in range(T):
        p, eh = t // 2, t % 2
        qh = QhT[eh * 64:(eh + 1) * 64, p, :]
        kh = KhT[eh * 64:(eh + 1) * 64, p, :]
        for qq in range(4):
            ctx = psum_ctx.tile([DK + 1, 512], F32, tag="ctx")
            for g in range(16 // KCG):
                s = psum_s.tile([128, KCG, 512], F32, tag="s")
                for u in range(KCG):
                    kc = KCG * g + u
                    nc.tensor.matmul(
                        s[:, u, :],
                        lhsT=kh[:, kc * 128:(kc + 1) * 128],
                        rhs=qh[:, qq * 512:(qq + 1) * 512],
                        start=True,
                        stop=True,
                    )
                pr = stage.tile([128, KCG, 512], BF16, tag="pr", bufs=3)
                nc.scalar.activation(pr[:], s[:], AF.Exp, scale=0.125)
                for u in range(KCG):
                    kc = KCG * g + u
                    nc.tensor.matmul(
                        ctx[:],
                        lhsT=Vh[:, t, kc, :],
                        rhs=pr[:, u, :],
                        start=(kc == 0),
                        stop=(kc == 15),
                    )
            # normalize by softmax sums (ctx row 64) and reorg into cbT:
            # cbT[c*64+d' , t, :, r] = ctx[d', (r*16+c)-qq*512] * rsum
            rsum = stage.tile([1, 512], F32, tag="rsum")
            nc.vector.reciprocal(rsum[:], ctx[DK:DK + 1, :])
            rsumB = stage.tile([64, 512], F32, tag="rsumB")
            nc.gpsimd.partition_broadcast(rsumB[:], rsum[:])
            for ce in range(2):
                csrc = ctx[0:64, :].rearrange("p (r c) -> p c r", c=16)[:, ce::2, :]
                sc = rsumB[:].rearrange("p (r c) -> p c r", c=16)[:, ce::2, :]
                dst = cbT[ce * 64:(ce + 1) * 64, t, :, qq * 32:(qq + 1) * 32]
                nc.vector.tensor_tensor(out=dst, in0=csrc, in1=sc, op=ALU.mult)

    # ---- stage 4: output projection ----
    for t in range(T):
        for h in range(2):
            ps = psum_pp.tile([128, ROWS], F32, tag="pp")
            for j in range(NJ):
                nc.tensor.matmul(
                    ps[:],
                    lhsT=cbT[:, t, j, :],
                    rhs=Wbf["o"][:, j, h * 512:(h + 1) * 512],
                    start=(j == 0),
                    stop=(j == NJ - 1),
                )
            osb = stage.tile([128, 512], F32, tag="ostage")
            nc.vector.tensor_tensor(
                out=osb[:], in0=ps[:], in1=boB[:, h * 512:(h + 1) * 512], op=ALU.add
            )
            nc.sync.dma_start(
                out=out_d[t * 128:(t + 1) * 128, h * 512:(h + 1) * 512], in_=osb[:]
            )


_CACHE = {}


def build():
    if "nc" in _CACHE:
        return _CACHE["nc"]
    nc = bacc.Bacc(None, target_bir_lowering=False)
    with tile.TileContext(nc) as tc:
        import contextlib

        with contextlib.ExitStack() as ctx:
            pools = {
                "persist": ctx.enter_context(tc.tile_pool(name="persist", bufs=1)),
                "stage": ctx.enter_context(tc.tile_pool(name="stage", bufs=2)),
                "psum_pp": ctx.enter_context(
                    tc.tile_pool(name="psum_pp", bufs=2, space="PSUM")
                ),
                "psum_s": ctx.enter_context(
                    tc.tile_pool(name="psum_s", bufs=1, space="PSUM")
                ),
                "psum_ctx": ctx.enter_context(
                    tc.tile_pool(name="psum_ctx", bufs=2, space="PSUM")
                ),
            }
            _emit(nc, tc, pools)
    nc.compile()
    _CACHE["nc"] = nc
    return nc


def kernel(x, Wq, bq, Wk, bk, Wv, bv, Wo, bo, _trace=False, _tmpdir=None):
    x = np.ascontiguousarray(np.asarray(x, dtype=np.float32))
    full = {
        "Wq": np.ascontiguousarray(np.asarray(Wq, np.float32)),
        "bq": np.ascontiguousarray(np.asarray(bq, np.float32)),
        "Wk": np.ascontiguousarray(np.asarray(Wk, np.float32)),
        "bk": np.ascontiguousarray(np.asarray(bk, np.float32)),
        "Wv": np.ascontiguousarray(np.asarray(Wv, np.float32)),
        "bv": np.ascontiguousarray(np.asarray(bv, np.float32)),
        "Wo": np.ascontiguousarray(np.asarray(Wo, np.float32)),
        "bo": np.ascontiguousarray(np.asarray(bo, np.float32)),
    }
    B, S, Dm = x.shape
    assert (B, S, Dm) == (2, 2048, 1024), (B, S, Dm)

    nc = build()
    in_maps = []
    for core in range(N_CORES):
        b, blk = core // 4, core % 4
        m = dict(full)
        m["x"] = np.ascontiguousarray(x[b, blk * ROWS:(blk + 1) * ROWS, :])
        in_maps.append(m)

    res = run_bass_kernel_spmd(
        nc,
        in_maps,
        core_ids=list(range(N_CORES)),
        trace=_trace,
        tmpdir=_tmpdir,
    )
    out = np.empty((B, S, Dm), np.float32)
    for core in range(N_CORES):
        b, blk = core // 4, core % 4
        out[b, blk * ROWS:(blk + 1) * ROWS, :] = res.results[core]["out"]
    if _trace:
        return out, res
    return out


# revision 9
# speedup vs baseline: 2.1373x; 2.1373x over previous
"""MultiHeadAttention (no-transpose head reshape) on 8 TRN2 NeuronCores.

The reference reshapes [B,S,D] -> [B,H,S',dk] WITHOUT transposing, so
"head h" of batch b is exactly rows [128h, 128h+128) of x viewed as 2048
pseudo-tokens of dim 64: pseudo-token (r, c) of head-block t is
x[t*128+r] features [c*64, c*64+64).  The whole problem is data-parallel
over the 32 (b,h) pairs: each of 8 cores owns 4 head-blocks (512 rows) of
one batch, no communication needed.

Internally pseudo-tokens are enumerated C-MAJOR (k'' = c*128 + r), which
is legal because softmax just sums over all keys (any consistent
permutation of keys works, and the query permutation is undone in the
final reorg).  With that ordering the "V with ones column" chunks are
natural slices of V, and every reorg copy moves contiguous 128-element
runs.

Per-core pipeline (bf16 matmuls, f32 accumulation):
  1. x -> bf16 -> xT via PE transposes (warms up the PE).
  2. V = x@Wv+bv straight into the ones-padded Vno layout;
     QT/KT = (x@W)^T with weight chunks stationary.
  3. QhT/KhT per-head [64,2048] via contiguous DVE copies.
  4. Attention per head: scoresT[k,q] = KhT^T@QhT (PSUM) -> exp on ACT
     (1/8 scale fused) -> probsT bf16 -> ctxT[65,q] += Vno^T@probsT,
     row 64 of ctxT accumulates softmax denominators (ones column).
  5. Normalize (DVE divide) + reorg into cbT (out-proj lhsT layout).
  6. out = cb@Wo + bo.

PE accumulation chains are emitted pairwise-interleaved so consecutive
matmuls hit different PSUM banks (fill overlaps drain).
"""

import sys

if "/opt/trn_rl_repo" not in sys.path:
    sys.path.insert(0, "/opt/trn_rl_repo")

import numpy as np

import concourse.bacc as bacc
import concourse.mybir as mybir
import concourse.tile as tile
from concourse.bass_utils import run_bass_kernel_spmd
from concourse.masks import make_identity

F32 = mybir.dt.float32
BF16 = mybir.dt.bfloat16
AF = mybir.ActivationFunctionType
ALU = mybir.AluOpType

N_CORES = 8
D = 1024
ROWS = 512          # rows of x per core
T = 4               # head-blocks (= heads) per core
NJ = 8              # 128-feature chunks of D
DK = 64
S2 = 2048           # pseudo-sequence length per head
GROUPS = (3, 3, 3, 3, 2, 2)   # k-chunks per exp group (sums to 16)
DEBUG = False


def _interleave(*seqs):
    """Round-robin the callables in seqs (lists of thunks), call in order."""
    n = max(len(s) for s in seqs)
    for u in range(n):
        for s in seqs:
            if u < len(s):
                s[u]()


def _emit(nc, tc, pools):
    persist = pools["persist"]
    stage = pools["stage"]
    psum_s = pools["psum_s"]      # tag "s": [128,3,512] f32, bufs=2 (6 banks)
    psum_ctx = pools["psum_ctx"]  # tag "ctx": [65,512] f32, bufs=2 (2 banks)

    x_d = nc.dram_tensor("x", [ROWS, D], F32, kind="ExternalInput")
    w_d = {}
    b_d = {}
    for w in ("q", "k", "v", "o"):
        w_d[w] = nc.dram_tensor(f"W{w}", [D, D], F32, kind="ExternalInput")
        b_d[w] = nc.dram_tensor(f"b{w}", [D], F32, kind="ExternalInput")
    out_d = nc.dram_tensor("out", [ROWS, D], F32, kind="ExternalOutput")

    # ---- persistent SBUF tensors ----
    xT = persist.tile([128, NJ, ROWS], BF16, name="xT")
    Wbf = {w: persist.tile([128, NJ, D], BF16, name=f"W{w}bf") for w in ("v", "q", "k", "o")}
    QT = persist.tile([128, NJ, ROWS], BF16, name="QT")
    KT = persist.tile([128, NJ, ROWS], BF16, name="KT")
    Vno = persist.tile([128, T, 16, DK + 1], BF16, name="Vno")
    QhT = persist.tile([128, 2, S2], BF16, name="QhT")
    KhT = persist.tile([128, 2, S2], BF16, name="KhT")
    cbT = persist.tile([128, T, NJ, 128], BF16, name="cbT")
    bq_sb = persist.tile([128, NJ], F32, name="bq_sb")
    bk_sb = persist.tile([128, NJ], F32, name="bk_sb")
    bvB = persist.tile([128, D], F32, name="bvB")
    boB = persist.tile([128, D], F32, name="boB")
    ident = persist.tile([128, 128], BF16, name="ident")

    make_identity(nc, ident[:])

    # ---- biases (small, early) ----
    nc.sync.dma_start(out=bq_sb[:], in_=b_d["q"].ap().rearrange("(j p) -> p j", p=128))
    nc.sync.dma_start(out=bk_sb[:], in_=b_d["k"].ap().rearrange("(j p) -> p j", p=128))
    nc.sync.dma_start(out=bvB[0:1, :], in_=b_d["v"].ap().unsqueeze(0))
    nc.gpsimd.partition_broadcast(bvB[:], bvB[0:1, :])
    nc.sync.dma_start(out=boB[0:1, :], in_=b_d["o"].ap().unsqueeze(0))
    nc.gpsimd.partition_broadcast(boB[:], boB[0:1, :])

    # ones columns of Vno (overwritten below except column 64)
    nc.gpsimd.memset(Vno[:], 1.0)

    # ---- stage 0: x -> xT (PE transposes; also warms up the PE) ----
    for t in range(T):
        xs = stage.tile([128, D], F32, tag="xstage")
        nc.sync.dma_start(out=xs[:], in_=x_d[t * 128:(t + 1) * 128, :])
        xb = stage.tile([128, D], BF16, tag="xbf")
        nc.vector.tensor_copy(out=xb[:], in_=xs[:])
        for i in range(NJ):
            tp = psum_s.tile([128, 128], BF16, tag="s", name="tp")
            nc.tensor.transpose(tp[:], xb[:, i * 128:(i + 1) * 128], ident[:])
            nc.vector.tensor_copy(out=xT[:, i, t * 128:(t + 1) * 128], in_=tp[:])

    # ---- weights: DMA + DVE cast, in consumption order ----
    for w in ("v", "q", "k", "o"):
        for i in range(NJ):
            ws = stage.tile([128, D], F32, tag="wstage")
            nc.sync.dma_start(out=ws[:], in_=w_d[w][i * 128:(i + 1) * 128, :])
            nc.vector.tensor_copy(out=Wbf[w][:, i, :], in_=ws[:])

    # ---- stage 1: projections (chains emitted pairwise for PE overlap) ----
    def v_chain(t, h):
        ps = psum_s.tile([128, 3, 512], F32, tag="s", name="psv")

        def mm(i):
            return lambda: nc.tensor.matmul(
                ps[:, 0, :],
                lhsT=xT[:, i, t * 128:(t + 1) * 128],
                rhs=Wbf["v"][:, i, h * 512:(h + 1) * 512],
                start=(i == 0),
                stop=(i == NJ - 1),
            )

        def evict():
            nc.vector.tensor_tensor(
                out=Vno[:, t, h * 8:(h + 1) * 8, 0:DK],
                in0=ps[:, 0, :].rearrange("p (c d) -> p c d", c=8),
                in1=bvB[:, h * 512:(h + 1) * 512].rearrange("p (c d) -> p c d", c=8),
                op=ALU.add,
            )

        return [mm(i) for i in range(NJ)], evict

    def qk_chain(wname, bsb, dst, j):
        ps = psum_s.tile([128, 3, 512], F32, tag="s", name="psqk")

        def mm(i):
            return lambda: nc.tensor.matmul(
                ps[:, 0, :],
                lhsT=Wbf[wname][:, i, j * 128:(j + 1) * 128],
                rhs=xT[:, i, :],
                start=(i == 0),
                stop=(i == NJ - 1),
            )

        def evict():
            nc.vector.tensor_scalar(
                out=dst[:, j, :], in0=ps[:, 0, :],
                scalar1=bsb[:, j:j + 1], scalar2=None, op0=ALU.add,
            )

        return [mm(i) for i in range(NJ)], evict

    chains = []
    for t in range(T):
        for h in range(2):
            chains.append(v_chain(t, h))
    for j in range(NJ):
        chains.append(qk_chain("q", bq_sb, QT, j))
    for j in range(NJ):
        chains.append(qk_chain("k", bk_sb, KT, j))
    for a in range(0, len(chains), 2):
        mmsA, evA = chains[a]
        mmsB, evB = chains[a + 1]
        _interleave(mmsA, mmsB)
        evA()
        evB()

    # ---- stage 2: QhT/KhT reorg (contiguous DVE copies) ----
    # QhT[eh*64+d', p, c*128+r] = QT[(c%2)*64+d', c//2, t*128+r],  t=2p+eh
    for srcT, dsthT in ((QT, QhT), (KT, KhT)):
        for t in range(T):
            p, eh = t // 2, t % 2
            for ce in range(2):
                src = srcT[ce * 64:(ce + 1) * 64, :, t * 128:(t + 1) * 128]
                dst = dsthT[eh * 64:(eh + 1) * 64, p, :].rearrange(
                    "p (c r) -> p c r", c=16
                )[:, ce::2, :]
                nc.vector.tensor_copy(out=dst, in_=src)

    # ---- stage 3: attention (software-pipelined scores/ctx interleave) ----
    if DEBUG:
        dbg_ctx = nc.dram_tensor("dbg_ctx", [4, DK + 1, 512], F32, kind="ExternalOutput")
        dbg_pr = nc.dram_tensor("dbg_pr", [128, 3, 512], F32, kind="ExternalOutput")
        dbg_s = nc.dram_tensor("dbg_s", [128, 3, 512], F32, kind="ExternalOutput")
        dbg_rsum = nc.dram_tensor("dbg_rsum", [4, 64, 512], F32, kind="ExternalOutput")
    for t in range(T):
        p, eh = t // 2, t % 2
        qh = QhT[eh * 64:(eh + 1) * 64, p, :]
        kh = KhT[eh * 64:(eh + 1) * 64, p, :]
        for qq in range(4):
            ctx = psum_ctx.tile([DK + 1, 512], F32, tag="ctx")
            pend_ctx = []  # ctx-matmul thunks of the previous group
            kc0 = 0
            for gi, gsz in enumerate(GROUPS):
                s = psum_s.tile([128, 3, 512], F32, tag="s", name="satt")
                sco = []
                for u in range(gsz):
                    kc = kc0 + u

                    def mk_s(u=u, kc=kc, s=s):
                        nc.tensor.matmul(
                            s[:, u, :],
                            lhsT=kh[:, kc * 128:(kc + 1) * 128],
                            rhs=qh[:, qq * 512:(qq + 1) * 512],
                            start=True,
                            stop=True,
                        )

                    sco.append(mk_s)
                _interleave(sco, pend_ctx)
                if DEBUG and t == 0 and qq == 0 and gi == 0:
                    sf = stage.tile([128, 3, 512], F32, tag="prf")
                    nc.vector.tensor_copy(out=sf[:], in_=s[:])
                    nc.scalar.dma_start(out=dbg_s[:], in_=sf[:])
                pr = stage.tile([128, 3, 512], BF16, tag="pr", bufs=3)
                nc.scalar.activation(
                    pr[:, 0:gsz, :], s[:, 0:gsz, :], AF.Exp, scale=0.125
                )
                if DEBUG and t == 0 and qq == 0 and gi == 0:
                    prf = stage.tile([128, 3, 512], F32, tag="prf")
                    nc.vector.tensor_copy(out=prf[:], in_=pr[:])
                    nc.scalar.dma_start(out=dbg_pr[:], in_=prf[:])
                pend_ctx = []
                for u in range(gsz):
                    kc = kc0 + u

                    def mk_c(u=u, kc=kc, pr=pr):
                        nc.tensor.matmul(
                            ctx[:],
                            lhsT=Vno[:, t, kc, :],
                            rhs=pr[:, u, :],
                            start=(kc == 0),
                            stop=(kc == 15),
                        )

                    pend_ctx.append(mk_c)
                kc0 += gsz
            for c in pend_ctx:
                c()
            if DEBUG and t == 0:
                cf = stage.tile([DK + 1, 512], F32, tag="ctxf")
                nc.vector.tensor_copy(out=cf[:], in_=ctx[:])
                nc.scalar.dma_start(out=dbg_ctx[qq, :, :], in_=cf[:])

            # normalize by softmax sums (ctx row 64) and reorg into cbT:
            # cbT[(c%2)*64+d', t, c//2, r] = ctx[d', (c-4qq)*128+r] / sums
            sums = stage.tile([1, 512], F32, tag="sums")
            nc.vector.tensor_copy(out=sums[:], in_=ctx[DK:DK + 1, :])
            rsum = stage.tile([1, 512], F32, tag="rsum")
            nc.vector.reciprocal_approx_fast(out=rsum[:], in_=sums[:])
            rsumB = stage.tile([64, 512], F32, tag="rsumB")
            nc.gpsimd.partition_broadcast(rsumB[:], rsum[:])
            if DEBUG and t == 0:
                nc.scalar.dma_start(out=dbg_rsum[qq, :, :], in_=rsumB[:])
            for ce in range(2):
                csrc = ctx[0:64, :].rearrange("p (c r) -> p c r", c=4)[:, ce::2, :]
                sc = rsumB[:].rearrange("p (c r) -> p c r", c=4)[:, ce::2, :]
                dst = cbT[ce * 64:(ce + 1) * 64, t, 2 * qq:2 * qq + 2, :]
                nc.vector.tensor_tensor(out=dst, in0=csrc, in1=sc, op=ALU.mult)

    # ---- stage 4: output projection (pairwise-interleaved chains) ----
    def o_chain(t, h):
        ps = psum_s.tile([128, 3, 512], F32, tag="s", name="pso")

        def mm(j):
            return lambda: nc.tensor.matmul(
                ps[:, 0, :],
                lhsT=cbT[:, t, j, :],
                rhs=Wbf["o"][:, j, h * 512:(h + 1) * 512],
                start=(j == 0),
                stop=(j == NJ - 1),
            )

        def evict():
            osb = stage.tile([128, 512], F32, tag="ostage")
            nc.vector.tensor_tensor(
                out=osb[:], in0=ps[:, 0, :], in1=boB[:, h * 512:(h + 1) * 512],
                op=ALU.add,
            )
            nc.scalar.dma_start(
                out=out_d[t * 128:(t + 1) * 128, h * 512:(h + 1) * 512], in_=osb[:]
            )

        return [mm(j) for j in range(NJ)], evict

    ochains = [o_chain(t, h) for t in range(T) for h in range(2)]
    for a in range(0, len(ochains), 2):
        mmsA, evA = ochains[a]
        mmsB, evB = ochains[a + 1]
        _interleave(mmsA, mmsB)
        evA()
        evB()

    if DEBUG:
        for nm, sb in (("dbg_xT", xT), ("dbg_QT", QT), ("dbg_KT", KT),
                       ("dbg_Vno", Vno), ("dbg_QhT", QhT), ("dbg_KhT", KhT),
                       ("dbg_cbT", cbT)):
            dd = nc.dram_tensor(nm, list(sb.shape), BF16, kind="ExternalOutput")
            nc.scalar.dma_start(out=dd[:], in_=sb[:])


_CACHE = {}


def build():
    if "nc" in _CACHE:
        return _CACHE["nc"]
    nc = bacc.Bacc(None, target_bir_lowering=False)
    with tile.TileContext(nc) as tc:
        import contextlib

        with contextlib.ExitStack() as ctx:
            pools = {
                "persist": ctx.enter_context(tc.tile_pool(name="persist", bufs=1)),
                "stage": ctx.enter_context(tc.tile_pool(name="stage", bufs=2)),
                "psum_s": ctx.enter_context(
                    tc.tile_pool(name="psum_s", bufs=2, space="PSUM")
                ),
                "psum_ctx": ctx.enter_context(
                    tc.tile_pool(name="psum_ctx", bufs=2, space="PSUM")
                ),
            }
            _emit(nc, tc, pools)
    nc.compile()
    _CACHE["nc"] = nc
    return nc


def kernel(x, Wq, bq, Wk, bk, Wv, bv, Wo, bo, _trace=False, _tmpdir=None):
    x = np.ascontiguousarray(np.asarray(x, dtype=np.float32))
    full = {
        "Wq": np.ascontiguousarray(np.asarray(Wq, np.float32)),
        "bq": np.ascontiguousarray(np.asarray(bq, np.float32)),
        "Wk": np.ascontiguousarray(np.asarray(Wk, np.float32)),
        "bk": np.ascontiguousarray(np.asarray(bk, np.float32)),
        "Wv": np.ascontiguousarray(np.asarray(Wv, np.float32)),
        "bv": np.ascontiguousarray(np.asarray(bv, np.float32)),
        "Wo": np.ascontiguousarray(np.asarray(Wo, np.float32)),
        "bo": np.ascontiguousarray(np.asarray(bo, np.float32)),
    }
    B, S, Dm = x.shape
    assert (B, S, Dm) == (2, 2048, 1024), (B, S, Dm)

    nc = build()
    in_maps = []
    for core in range(N_CORES):
        b, blk = core // 4, core % 4
        m = dict(full)
        m["x"] = np.ascontiguousarray(x[b, blk * ROWS:(blk + 1) * ROWS, :])
        in_maps.append(m)

    res = run_bass_kernel_spmd(
        nc,
        in_maps,
        core_ids=list(range(N_CORES)),
        trace=_trace,
        tmpdir=_tmpdir,
    )
    out = np.empty((B, S, Dm), np.float32)
    for core in range(N_CORES):
        b, blk = core // 4, core % 4
        out[b, blk * ROWS:(blk + 1) * ROWS, :] = res.results[core]["out"]
    if _trace:
        return out, res
    return out
